# revision 11
# baseline (speedup 1.0000x reference)
"""HadLinear TRN2 kernel: out = fwht_1024blocks(x)/sqrt(1024) @ W.T

Math: fwht on 1024-blocks is x @ H_bd with H_bd = blockdiag(H_1024 x4),
H_1024 = H_8 (x) H_128 (Sylvester/natural order, index k = j*128 + p).
The 1/sqrt(1024) = 2^-5 scale is folded into H_128 (exact in bf16).

Sharding: data-parallel row shard of x (2048 rows/core). The host passes
x and W pre-transposed (pure layout change) so every device load is a
full-rate contiguous DMA; no DMA transposes. All DMAs are batched to
~1MB via 3D access patterns (small transfers are descriptor-dominated:
256KB runs at ~55% of peak).

Per core:
  Phase A: cast xT k-chunks to bf16 (DVE), V2[j]/V2[j+4] = H128 @ x[j]
           +/- H128 @ x[j+4] accumulated on PE (first H8 butterfly stage
           folded into PSUM accumulation via +/-H128 constants), ACT
           evicts V2 to bf16 SBUF, remaining 2 butterfly stages in bf16
           on GpSimd/DVE emitting bf16 A-tiles (= xh^T) resident in SBUF.
  Phase B: C[m,n] = sum_kt A[kt,m-sub].T @ Wbf[kt,n-strip] accumulated in
           PSUM over the 32 k-tiles. m is split in two halves so phase A
           of half 1 overlaps phase B of half 0 on the PE. During half 0,
           W^T f32 streams in 1MB chunks, is cast to bf16 (split DVE/ACT)
           and also written to a DRAM bf16 scratch; half 1 streams the
           scratch directly with zero cast work.

Self-contained: hardcodes shapes B=4, S=4096, D_in=D_out=4096, 8 cores.
"""

import numpy as np
import ml_dtypes

import concourse.bacc as bacc
import concourse.mybir as mybir
import concourse.tile as tile
from concourse.bass_utils import run_bass_kernel_spmd

P = 128
N_CORES = 8
B_FULL, S_FULL, D = 4, 4096, 4096
M_FULL = B_FULL * S_FULL          # 16384 rows total
M_CORE = M_FULL // N_CORES        # 2048 rows per core
HAD = 1024                        # hadamard block
NBLK = D // HAD                   # 4 blocks of 1024
ASTRIP = 256                      # phase A m-strip width
NSTRIP = 512                      # phase B out-feature strip width
HALF_M = M_CORE // 2              # 1024 rows per half
KT = D // P                       # 32 k-tiles
WCH = 4                           # k-tiles per W chunk (1MB f32 DMAs)


def _h128_np():
    """H_128 (natural order) scaled by 1/sqrt(1024) = 2^-5; exact in bf16."""
    h = np.array([[(-1.0) ** bin(i & j).count("1") for j in range(P)]
                  for i in range(P)])
    return (h / 32.0).astype(ml_dtypes.bfloat16)


def build_nc():
    f32, bf16 = mybir.dt.float32, mybir.dt.bfloat16
    nc = bacc.Bacc(None, target_bir_lowering=False, debug=False)

    xt = nc.declare_dram_parameter("xt", [D, M_CORE], f32, isOutput=False)
    wt = nc.declare_dram_parameter("wt", [D, D], f32, isOutput=False)
    h = nc.declare_dram_parameter("h", [2 * P, P], bf16, isOutput=False)
    y = nc.declare_dram_parameter("y", [M_CORE, D], f32, isOutput=True)

    ms_per_half = HALF_M // ASTRIP            # 4 m-strips per half
    grp_per_half = HALF_M // P                # 8 output-row groups per half
    ns_total = D // NSTRIP                    # 8 out strips
    nch = KT // WCH                           # 8 W chunks per n-strip

    with tile.TileContext(nc) as tc:
        with (
            tc.tile_pool(name="dram", bufs=1, space="DRAM") as dramp,
            tc.tile_pool(name="const", bufs=1) as constp,
            tc.tile_pool(name="apool", bufs=22) as apool,
            tc.tile_pool(name="xs", bufs=3) as xsp,
            tc.tile_pool(name="xb", bufs=2) as xbp,
            tc.tile_pool(name="bfly", bufs=4) as bflyp,
            tc.tile_pool(name="wst", bufs=2) as wstp,
            tc.tile_pool(name="wbf", bufs=12) as wbfp,
            tc.tile_pool(name="outp", bufs=2) as outp,
            tc.tile_pool(name="psV", bufs=3, space="PSUM") as psV,
            tc.tile_pool(name="psC", bufs=5, space="PSUM") as psC,
        ):
            h128p = constp.tile([P, P], bf16, tag="hp", name="h128p")
            nc.sync.dma_start(out=h128p[:], in_=h[0:P, :])
            h128n = constp.tile([P, P], bf16, tag="hn", name="h128n")
            nc.sync.dma_start(out=h128n[:], in_=h[P:2 * P, :])

            # bf16 W^T scratch, written during half 0's pass
            wscr = dramp.tile([D, D], bf16, name="wscr")

            a_tiles = {}

            def phase_a(half):
                for msl in range(ms_per_half):
                    ms = half * ms_per_half + msl
                    m0 = ms * ASTRIP
                    for blk in range(NBLK):
                        # two 0.5MB DMAs + bf16 casts (DVE)
                        xb = xbp.tile([P, 8, ASTRIP], bf16, tag="xb",
                                      name=f"xb_{ms}_{blk}")
                        for hf in range(2):
                            xs = xsp.tile([P, 4, ASTRIP], f32, tag="xs",
                                          name=f"xs_{ms}_{blk}_{hf}")
                            k0 = blk * HAD + hf * 4 * P
                            src = xt[k0:k0 + 4 * P, m0:m0 + ASTRIP]
                            nc.sync.dma_start(
                                out=xs[:],
                                in_=src.rearrange("(j p) m -> p j m", p=P))
                            nc.vector.tensor_copy(
                                out=xb[:, hf * 4:hf * 4 + 4, :], in_=xs[:])
                        # first H8 butterfly stage (j, j+4) folded into PE
                        # PSUM accumulation: V2[j] = H@x[j] + H@x[j+4],
                        # V2[j+4] = H@x[j] - H@x[j+4] (via -H constant);
                        # the pair shares one PSUM bank so a single ACT op
                        # evicts both to bf16. Remaining two stages run as
                        # wide strided ops in bf16 on DVE/GpSimd.
                        et = bflyp.tile([P, 8, ASTRIP], bf16, tag="bf",
                                        name=f"e_{ms}_{blk}")
                        for j in range(4):
                            v2 = psV.tile([P, 2, ASTRIP], f32, tag="V",
                                          name=f"v_{ms}_{blk}_{j}")
                            for half_idx, hsec in ((0, h128p), (1, h128n)):
                                nc.tensor.matmul(
                                    v2[:, half_idx, :], lhsT=h128p[:],
                                    rhs=xb[:, j, :],
                                    start=True, stop=False)
                                nc.tensor.matmul(
                                    v2[:, half_idx, :], lhsT=hsec[:],
                                    rhs=xb[:, j + 4, :],
                                    start=False, stop=True)
                            nc.scalar.copy(out=et[:, j:j + 5:4, :], in_=v2[:])
                        s2t = bflyp.tile([P, 8, ASTRIP], bf16, tag="bf",
                                         name=f"s2_{ms}_{blk}")
                        nc.vector.tensor_add(
                            out=s2t[:, 0:2, :], in0=et[:, 0:2, :], in1=et[:, 2:4, :])
                        nc.gpsimd.tensor_sub(
                            out=s2t[:, 2:4, :], in0=et[:, 0:2, :], in1=et[:, 2:4, :])
                        nc.gpsimd.tensor_add(
                            out=s2t[:, 4:6, :], in0=et[:, 4:6, :], in1=et[:, 6:8, :])
                        nc.gpsimd.tensor_sub(
                            out=s2t[:, 6:8, :], in0=et[:, 4:6, :], in1=et[:, 6:8, :])
                        ablk = apool.tile([P, 8, ASTRIP], bf16, tag="A",
                                          name=f"A_{ms}_{blk}")
                        nc.vector.tensor_add(
                            out=ablk[:, 0:8:2, :], in0=s2t[:, 0:8:2, :],
                            in1=s2t[:, 1:8:2, :])
                        nc.vector.tensor_sub(
                            out=ablk[:, 1:8:2, :], in0=s2t[:, 0:8:2, :],
                            in1=s2t[:, 1:8:2, :])
                        a_tiles[(blk, ms)] = ablk

            def lhsT_of(gg, kt):
                ms, sub = divmod(gg, ASTRIP // P)
                blk, j = divmod(kt, 8)
                return a_tiles[(blk, ms)][:, j, sub * P:(sub + 1) * P]

            def phase_b(half):
                for ns in range(ns_total):
                    n0 = ns * NSTRIP
                    wbf = [None] * nch
                    for ch in range(nch):
                        k0 = ch * WCH * P
                        wb = wbfp.tile([P, WCH, NSTRIP], bf16, tag="wbf",
                                       name=f"wbf_{half}_{ns}_{ch}")
                        if half == 0:
                            # 1MB f32 load, cast (DVE/ACT alternating),
                            # write bf16 scratch for half 1
                            wst = wstp.tile([P, WCH, NSTRIP], f32, tag="wst",
                                            name=f"wst_{ns}_{ch}")
                            src = wt[k0:k0 + WCH * P, n0:n0 + NSTRIP]
                            nc.sync.dma_start(
                                out=wst[:],
                                in_=src.rearrange("(c p) n -> p c n", p=P))
                            if ch % 2 == 0:
                                nc.vector.tensor_copy(out=wb[:], in_=wst[:])
                            else:
                                nc.scalar.copy(out=wb[:], in_=wst[:])
                            dst = wscr[k0:k0 + WCH * P, n0:n0 + NSTRIP]
                            nc.sync.dma_start(
                                out=dst.rearrange("(c p) n -> p c n", p=P),
                                in_=wb[:])
                        else:
                            src = wscr[k0:k0 + WCH * P, n0:n0 + NSTRIP]
                            nc.sync.dma_start(
                                out=wb[:],
                                in_=src.rearrange("(c p) n -> p c n", p=P))
                        wbf[ch] = wb
                    for st in range(grp_per_half // 4):
                        grps = [st * 4 + i for i in range(4)]
                        cps = {g: psC.tile([P, NSTRIP], f32, tag="C",
                                           name=f"c_{half}_{ns}_{g}")
                               for g in grps}
                        for kt in range(KT):
                            for g in grps:
                                gg = half * grp_per_half + g
                                nc.tensor.matmul(
                                    cps[g][:],
                                    lhsT=lhsT_of(gg, kt),
                                    rhs=wbf[kt // WCH][:, kt % WCH, :],
                                    start=(kt == 0), stop=(kt == KT - 1),
                                )
                        # evict the whole set into one tile -> one 1MB store
                        cout = outp.tile([P, 4, NSTRIP], f32, tag="out",
                                         name=f"co_{half}_{ns}_{st}")
                        for i, g in enumerate(grps):
                            nc.scalar.copy(out=cout[:, i, :], in_=cps[g][:])
                        r0 = (half * grp_per_half + grps[0]) * P
                        dst = y[r0:r0 + 4 * P, n0:n0 + NSTRIP]
                        nc.sync.dma_start(
                            out=dst.rearrange("(c p) n -> p c n", p=P),
                            in_=cout[:])

            phase_a(0)
            phase_b(0)
            phase_a(1)
            phase_b(1)

    nc.compile()
    return nc


_CACHE = {}


def _get_nc():
    if "nc" not in _CACHE:
        _CACHE["nc"] = build_nc()
    return _CACHE["nc"]


def run(x, weight, trace=False):
    assert x.shape == (B_FULL, S_FULL, D) and weight.shape == (D, D)
    nc = _get_nc()
    xf = np.asarray(x, dtype=np.float32).reshape(M_FULL, D)
    wtv = np.ascontiguousarray(np.asarray(weight, dtype=np.float32).T)
    h1 = _h128_np()
    hh = np.concatenate([h1, -h1], axis=0)
    in_maps = [
        {"xt": np.ascontiguousarray(xf[c * M_CORE:(c + 1) * M_CORE].T),
         "wt": wtv, "h": hh}
        for c in range(N_CORES)
    ]
    res = run_bass_kernel_spmd(nc, in_maps, core_ids=list(range(N_CORES)),
                               trace=trace)
    yv = np.concatenate([r["y"] for r in res.results], axis=0)
    return yv.reshape(B_FULL, S_FULL, D), res


def kernel(x, weight):
    return run(x, weight)[0]
